# revision 20
# baseline (speedup 1.0000x reference)
"""AnyLoc VLAD (vq_codebook) Trainium2 kernel, 8-core data parallel. v3.

Reference computation (per image, N=1024 patches, K=64 clusters, D=1536):
  descs_n = l2norm(query_descs)                 # row-normalize descriptors
  labels  = argmax_k(descs_n . l2norm(centers)) # hard assignment
  sum_d_k = sum_{n: label=k} descs_n            # per-cluster sum
  un_vlad = sum_d_k - count_k * centers_k
  vlad    = l2norm_rows(un_vlad); flatten; l2norm

Sharding: data-parallel over the batch axis, 4 images per NeuronCore; each
core holds the whole (tiny) codebook; host concatenates the per-core
outputs (no collectives needed).

The kernel is DMA-wire-bound (12.6 MB of fp8 descriptors per core at
~330 GB/s aggregate across the 16 DMA engines), so the structure keeps the
two hardware-DGE queues (sync + scalar) streaming continuously and hides
all compute under them:

  - host pre-normalizes descriptors (fp8 at x64 scale) and ships TWO
    layouts: natural patch-major pair tiles (agg rhs) and a DoubleRow-
    packed transposed layout (sims rhs). Every DMA row is one contiguous
    6 KB per-partition packet.
  - the pipeline runs in HALF-IMAGE slots (512 patches): per slot one tsp
    DMA (sync queue) + one nat DMA (scalar queue), 6 DoubleRow fp8 sims
    matmuls (codebook stationary -> simsT [64,512] psum, 1 bank), an ACT
    copy to bf16, 4 PE transposes back to patch-major (one shared bank;
    `start` only on the first write so a lazy hardware zero cannot wipe
    earlier chunks), one segmented DVE row-max and one is_ge against a
    stride-0 broadcast -> exact 1.0 one-hot fp8.
  - each slot's aggregation (6 DoubleRow matmuls + 2 one-column counts
    matmuls, accumulated per image) is deferred into the NEXT slot's
    stream so TensorE alternates sims_s+1 / agg_s with no idle window.
  - finalize: un_vlad = (-64*counts)*centers + agg in one pass split
    halves across DVE and GPSIMD (reads the agg psum directly, freeing
    banks); row norms via ACT Square+accum; global norm = sqrt(#nonzero
    rows) via a ones-matmul; final scale split ACT/GPSIMD; bf16 output
    (host upcasts) halves the write traffic; out DMAs ride the sync queue
    whose input traffic ends first.
  - PSUM budget exactly 8 banks: 2 sims + 2 transpose + 1 counts + 3 agg.

Toolchain workarounds: this walrus build accepts only one sync wait per
instruction, so Tile's tail drain is re-spread across per-engine drains
and a post-pass hoists surplus waits onto no-op carriers.
"""

import os
import sys

import numpy as np

for _p in ("/opt/trn_rl_repo", "/root/.axon_site/_ro/trn_rl_repo"):
    if os.path.isdir(_p) and _p not in sys.path:
        sys.path.insert(0, _p)

from contextlib import ExitStack

import ml_dtypes
import bass_rust
import concourse.bass as bass
import concourse.tile as tile
from concourse import bass_isa, library_config, mybir
from concourse.bass_utils import run_bass_kernel_spmd

B, N, K, D = 32, 1024, 64, 1536
NCORES = 8
IMGS = B // NCORES  # images per core
P = 128
NPAIR = 4   # patch chunk-pairs per image (N = NPAIR*256)
CP = 6      # feature chunk-pairs (D = CP*256)
JJ = D // 512  # agg column blocks
DH = D // 2    # finalize half split
BF16 = mybir.dt.bfloat16
FP8 = mybir.dt.float8e4
F32 = mybir.dt.float32
NP_BF16 = ml_dtypes.bfloat16
NP_FP8 = ml_dtypes.float8_e4m3
Alu = mybir.AluOpType
Act = mybir.ActivationFunctionType
DR = mybir.MatmulPerfMode.DoubleRow
EPS = 1e-12


def _patch_tile_drain():
    """This walrus build only accepts ONE sync wait per instruction; Tile's
    tail drain aggregates every outstanding semaphore wait onto a single
    Drain. Spread the waits across extra per-engine drains (all still
    before the end-of-kernel barrier, so semantics are unchanged)."""
    if getattr(tile.TileContext, "_vlad_drain_patched", False):
        return
    from concourse.vector_clock import ScopedClock

    def patched(self, tick_clock, wait_clock):
        nc = self.nc
        probe = nc.sync.drain()
        wait_clock.add_sem_waits(
            probe.ins, ScopedClock({None: tick_clock.global_clock})
        )
        si = probe.ins.sync_info
        waits = list(si.on_wait) if si is not None else []
        upds = list(si.on_update) if si is not None else []
        probe.ins.sync_info = bass_rust.SyncInfo(on_wait=waits[:1], on_update=upds)
        engines = [nc.scalar, nc.vector, nc.tensor, nc.gpsimd, nc.sync]
        for i, w in enumerate(waits[1:]):
            d = engines[i % len(engines)].drain()
            dsi = d.ins.sync_info
            du = list(dsi.on_update) if dsi is not None else []
            d.ins.sync_info = bass_rust.SyncInfo(on_wait=[w], on_update=du)
        nc.all_engine_barrier()
        popped = nc._tile_sem_poison_stack.pop()
        assert popped is self._sem_poison
        nc.clear_and_free_semaphores(list(self.sems.allocated().values()))

    tile.TileContext._drain_and_barrier = patched
    tile.TileContext._vlad_drain_patched = True


def _split_multi_waits(nc):
    """Walrus here accepts only one sync wait per instruction. Hoist surplus
    waits onto no-op carrier instructions inserted just before, on the same
    engine (safe: same engine executes in order, so all waits still complete
    before the original instruction issues)."""
    n_new = 0
    for _bbname, bassbb in list(nc.bb_map.items()):
        bb = bassbb.bb
        out = []
        changed = False
        for ins in bb.instructions:
            si = getattr(ins, "sync_info", None)
            waits = list(si.on_wait) if si is not None else []
            if len(waits) > 1:
                changed = True
                for w in waits[:-1]:
                    n_new += 1
                    nop = mybir.InstNoOp(
                        name=f"{ins.name}-wsplit{n_new}",
                        sync_info=mybir.SyncInfo(on_wait=[w], on_update=[]),
                        bass_nofuse=True,
                        engine=ins.engine,
                    )
                    nc.register_instruction(nop)
                    out.append(nop)
                ins.sync_info = bass_rust.SyncInfo(
                    on_wait=[waits[-1]], on_update=list(si.on_update)
                )
            out.append(ins)
        if changed:
            bb.instructions = out
    return n_new


def _slot_geom(npair):
    """Half-image slotting: S slots per image, pps chunk-pairs per slot."""
    S = 2 if npair % 2 == 0 and npair >= 2 else 1
    pps = npair // S
    nsl = pps * 2 * P  # patch columns per slot
    return S, pps, nsl


def build_nc(imgs=IMGS, npair=NPAIR):
    """Build the per-core Bass graph. `imgs`/`npair` shrinkable for sim."""
    _patch_tile_drain()
    S, pps, nsl = _slot_geom(npair)
    nch_s = 2 * pps  # 128-patch chunks per slot

    nc = bass.Bass("TRN2", target_bir_lowering=False, debug=False)
    # natural pair tiles: row (slot, p) = 6KB [cp, q, d] flat, where
    # element (cp, q, d) = desc[chunk 2*(slot_pairbase+cp)+q, patch p, d]
    descsn_e = nc.dram_tensor("descsn", [imgs * S * P, pps * 2 * D], FP8,
                              kind="ExternalInput")
    # DoubleRow-packed transpose: row (slot, p) = 6KB [c, q, n] flat with
    # element (c, q, n) = desc[b, slot_n0 + n, 256c+128q+p]
    descst_e = nc.dram_tensor("descst", [imgs * S * P, CP * 2 * nsl], FP8,
                              kind="ExternalInput")
    # codebook, same DoubleRow packing: row (c, p) = [q, k] = cnorm64[k, 256c+128q+p]
    cnt2_e = nc.dram_tensor("cnt2", [CP * P, 2 * K], FP8, kind="ExternalInput")
    cen_e = nc.dram_tensor("cen", [K, D], BF16, kind="ExternalInput")
    ident_e = nc.dram_tensor("ident", [K, K], BF16, kind="ExternalInput")
    out_e = nc.dram_tensor("out", [imgs, K * D], BF16, kind="ExternalOutput")

    with tile.TileContext(nc) as tc:
        with ExitStack() as ctx:
            consts = ctx.enter_context(tc.tile_pool(name="consts", bufs=1))
            tspp = ctx.enter_context(tc.tile_pool(name="tspp", bufs=6))
            natp = ctx.enter_context(tc.tile_pool(name="natp", bufs=6))
            simsbp = ctx.enter_context(tc.tile_pool(name="simsbp", bufs=2))
            asnp = ctx.enter_context(tc.tile_pool(name="asnp", bufs=2))
            mxp = ctx.enter_context(tc.tile_pool(name="mxp", bufs=2))
            uvp = ctx.enter_context(tc.tile_pool(name="uvp", bufs=2))
            sqp = ctx.enter_context(tc.tile_pool(name="sqp", bufs=2))
            vfinp = ctx.enter_context(tc.tile_pool(name="vfinp", bufs=2))
            finp = ctx.enter_context(tc.tile_pool(name="finp", bufs=16))
            simsps = ctx.enter_context(
                tc.tile_pool(name="simsps", bufs=2, space="PSUM"))
            transps = ctx.enter_context(
                tc.tile_pool(name="transps", bufs=2, space="PSUM"))
            cntps = ctx.enter_context(
                tc.tile_pool(name="cntps", bufs=1, space="PSUM"))
            aggps = ctx.enter_context(
                tc.tile_pool(name="aggps", bufs=1, space="PSUM"))

            cnt_sb = consts.tile([P, CP, 2, K], FP8)
            nc.sync.dma_start(
                out=cnt_sb,
                in_=cnt2_e.ap().rearrange("(c p) (q k) -> p c q k", c=CP, q=2),
            )
            ident_sb = consts.tile([K, K], BF16)
            nc.sync.dma_start(out=ident_sb, in_=ident_e.ap())
            onesc = consts.tile([P, 2, 1], FP8)
            nc.vector.memset(onesc, 1.0)
            # finalize-only consts load lazily in slot 0's shadow
            late = {}

            def _late():
                if not late:
                    cen_sb = consts.tile([K, D], BF16)
                    nc.scalar.dma_start(out=cen_sb, in_=cen_e.ap())
                    onesg = consts.tile([K, K], BF16)
                    nc.vector.memset(onesg, 1.0)
                    late.update(cen_sb=cen_sb, onesg=onesg)
                return late

            img_agg = {}

            def emit_agg(s):
                """Aggregate slot s's one-hot (deferred one slot)."""
                b, sj = s["b"], s["sj"]
                if sj == 0:
                    img_agg[b] = dict(
                        agg=aggps.tile([K, JJ, 512], F32, tag="agg", name="agg"),
                        counts=cntps.tile([K, 1], F32, tag="cnt", name="counts"),
                    )
                ia = img_agg[b]
                for cp in range(pps):
                    for jj in range(JJ):
                        nc.tensor.matmul(
                            ia["agg"][:, jj, :],
                            lhsT=s["asn"][:, 2 * cp:2 * cp + 2, :],
                            rhs=s["nat"][:, cp, :, jj * 512:(jj + 1) * 512],
                            start=(sj == 0 and cp == 0), stop=False,
                            perf_mode=DR, skip_group_check=True,
                        )
                for cp in range(pps):
                    nc.tensor.matmul(
                        ia["counts"], lhsT=s["asn"][:, 2 * cp:2 * cp + 2, :],
                        rhs=onesc,
                        start=(sj == 0 and cp == 0),
                        stop=(sj == S - 1 and cp == pps - 1),
                        perf_mode=DR, skip_group_check=True,
                    )

            def emit_fin1(b):
                """Image finalize part 1: un_vlad + row norm accumulation.
                GPSIMD cannot touch PSUM, so the -64*counts*centers term is
                folded into the agg psum by a diagonal-lhsT matmul (TensorE
                has slack in this DMA-bound regime). un_vlad is then copied
                to SBUF in ACT/DVE halves, releasing the agg banks early so
                the next image's aggregation never stalls on this finalize;
                the row-norm accumulation also runs as parallel halves."""
                lc = _late()
                ia = img_agg.pop(b)
                # diag = ident * counts * -64 in ONE DVE op (fewer hops in
                # the drain-critical finalize chain)
                diag = finp.tile([K, K], BF16, tag="diag")
                nc.vector.tensor_scalar(
                    diag, ident_sb, scalar1=ia["counts"], scalar2=-64.0,
                    op0=Alu.mult, op1=Alu.mult)
                for jj in range(JJ):
                    nc.tensor.matmul(
                        ia["agg"][:, jj, :], lhsT=diag,
                        rhs=lc["cen_sb"][:, jj * 512:(jj + 1) * 512],
                        start=False, stop=(jj == JJ - 1),
                        skip_group_check=True,
                    )
                aggflat = ia["agg"].rearrange("k a b -> k (a b)")
                uv = uvp.tile([K, D], F32, tag="uv")
                nc.scalar.mul(uv[:, 0:DH], aggflat[:, 0:DH], 1.0)
                nc.vector.tensor_scalar_mul(uv[:, DH:D], aggflat[:, DH:D], 1.0)
                sq = sqp.tile([K, D], FP8, tag="sq")
                r2a = finp.tile([K, 1], F32, tag="r2a")
                nc.scalar.activation(sq[:, 0:DH], uv[:, 0:DH], Act.Square,
                                     accum_out=r2a)
                r2b = finp.tile([K, 1], F32, tag="r2b")
                nc.vector.scalar_tensor_tensor(
                    out=sq[:, DH:D], in0=uv[:, DH:D], scalar=1.0,
                    in1=uv[:, DH:D], op0=Alu.mult, op1=Alu.mult,
                    accum_out=r2b,
                )
                r2 = finp.tile([K, 1], F32, tag="r2")
                nc.vector.tensor_tensor(r2, r2a, r2b, op=Alu.add)
                u = finp.tile([K, 1], F32, tag="u")
                nc.scalar.sqrt(u, r2)
                # nonzero-row gate from r2 (not u): lets the global-norm
                # branch (g/sg/ginv) run in parallel with um/invu
                sgate = finp.tile([K, 1], BF16, tag="sgate")
                nc.vector.tensor_scalar(
                    sgate, r2, scalar1=1e30, scalar2=1.0,
                    op0=Alu.mult, op1=Alu.min,
                )
                # G = sum_k gate_k via a ones-matmul into the agg pool: its
                # banks freed at the uv copy just above, and this avoids
                # coupling the transpose-pool rotation to the finalize
                g_ps = aggps.tile([K, 1], F32, tag="agg", name="g_ps")
                nc.tensor.matmul(g_ps, lhsT=lc["onesg"], rhs=sgate,
                                 start=True, stop=True, skip_group_check=True)
                sg = finp.tile([K, 1], F32, tag="sg")
                nc.scalar.sqrt(sg, g_ps)
                return dict(b=b, uv=uv, u=u, sg=sg)

            def emit_fin2(f):
                """Image finalize part 2: scales + output DMA."""
                um = finp.tile([K, 1], F32, tag="um")
                nc.vector.tensor_scalar_max(um, f["u"], EPS)
                invu = finp.tile([K, 1], F32, tag="invu")
                nc.vector.reciprocal(invu, um)
                ginv = finp.tile([K, 1], F32, tag="ginv")
                nc.vector.reciprocal(ginv, f["sg"])
                tot = finp.tile([K, 1], F32, tag="tot")
                nc.vector.tensor_mul(tot, invu, ginv)
                # final scale from the SBUF un_vlad, halves on ACT/DVE;
                # each half's output DMA starts as soon as it is ready
                vfin = vfinp.tile([K, D], BF16, tag="vfin")
                out_kd = out_e.ap()[f["b"]].rearrange("(k d) -> k d", k=K)
                nc.scalar.mul(vfin[:, 0:DH], f["uv"][:, 0:DH], tot)
                nc.sync.dma_start(out=out_kd[:, 0:DH], in_=vfin[:, 0:DH])
                nc.vector.tensor_scalar_mul(
                    vfin[:, DH:D], f["uv"][:, DH:D], tot)
                nc.sync.dma_start(out=out_kd[:, DH:D], in_=vfin[:, DH:D])

            # The PE drops to a low p-state after any idle gap and needs
            # ~3us of continuous execution to reach full clock, so the slot
            # pipeline is staged so that EVERY tensor instruction's inputs
            # are ready before the engine reaches it: transposes run one
            # slot behind sims (their ACT copy finished last stream), and
            # aggregation runs two slots behind (its one-hot finished last
            # stream). TensorE then never waits mid-run.
            def emit_transp_assign(s):
                trT = transps.tile([P, nch_s, K], BF16, tag="tr", name="trT")
                for ch in range(nch_s):
                    nc.tensor.matmul(
                        trT[:, ch, :],
                        lhsT=s["simsSb"][:, ch * P:(ch + 1) * P],
                        rhs=ident_sb, is_transpose=True,
                        start=(ch == 0), stop=(ch == nch_s - 1),
                        skip_group_check=True,
                    )
                mx = mxp.tile([P, nch_s], F32, tag="mx", name="mx")
                nc.vector.tensor_reduce(
                    mx, trT, axis=mybir.AxisListType.X, op=Alu.max)
                asn = asnp.tile([P, nch_s, K], FP8, tag="asn", name="asn")
                nc.vector.scalar_tensor_tensor(
                    out=asn, in0=trT, scalar=1.0,
                    in1=mx[:, :, None].broadcast_to([P, nch_s, K]),
                    op0=Alu.mult, op1=Alu.is_ge,
                )
                s["asn"] = asn

            prev1 = None  # slot awaiting transpose+assign (1 behind)
            prev2 = None  # slot awaiting aggregation (2 behind)
            fin1_pending = None
            for t in range(imgs * S):
                b, sj = divmod(t, S)
                tsp = tspp.tile([P, CP, 2, nsl], FP8, tag="tsp")
                nc.sync.dma_start(
                    out=tsp,
                    in_=descst_e.ap()[t * P:(t + 1) * P, :]
                    .rearrange("p (c q n) -> p c q n", c=CP, q=2),
                )
                nat = natp.tile([P, pps, 2, D], FP8, tag="nat")
                nc.scalar.dma_start(
                    out=nat,
                    in_=descsn_e.ap()[t * P:(t + 1) * P, :]
                    .rearrange("p (c q d) -> p c q d", c=pps, q=2),
                )
                # sims^T: codebook-stationary DoubleRow accumulation
                simsT = simsps.tile([K, nsl], F32, tag="sims")
                for c in range(CP):
                    nc.tensor.matmul(
                        simsT, lhsT=cnt_sb[:, c], rhs=tsp[:, c],
                        start=(c == 0), stop=(c == CP - 1),
                        perf_mode=DR, skip_group_check=True,
                    )
                simsSb = simsbp.tile([K, nsl], BF16, tag="simsb")
                nc.scalar.mul(simsSb, simsT, 1.0)
                if prev1 is not None:
                    emit_transp_assign(prev1)
                if prev2 is not None:
                    emit_agg(prev2)
                    if prev2["sj"] == S - 1:
                        fin1_pending = emit_fin1(prev2["b"])
                # fin2 at stream end: the fin1 ACT round-trip has completed
                # and the agg banks free before the next image rotates in
                if fin1_pending is not None and (
                        prev2 is None or prev2["sj"] == S - 1):
                    emit_fin2(fin1_pending)
                    fin1_pending = None
                prev2 = prev1
                prev1 = dict(b=b, sj=sj, nat=nat, simsSb=simsSb)
            # drain: transpose/assign of the last slot, then the last two
            # slots' aggregation and the final image's finalize
            emit_transp_assign(prev1)
            if prev2 is not None:
                emit_agg(prev2)
                if prev2["sj"] == S - 1:
                    emit_fin2(emit_fin1(prev2["b"]))
            emit_agg(prev1)
            if prev1["sj"] == S - 1:
                emit_fin2(emit_fin1(prev1["b"]))

    _split_multi_waits(nc)
    return nc


def prep_inputs(query_descs, c_centers, imgs=IMGS, npair=NPAIR, ncores=NCORES):
    """Host-side layout prep shared by kernel() and tests."""
    S, pps, nsl = _slot_geom(npair)
    NN = npair * 2 * P
    qd = np.ascontiguousarray(query_descs, dtype=np.float32)
    cc = np.ascontiguousarray(c_centers, dtype=np.float32)
    # normalized descriptors at x64 scale (sweet spot for fp8e4m3); the
    # x64 factors cancel in argmax and under the downstream l2norms
    nrm = np.maximum(np.linalg.norm(qd, axis=-1, keepdims=True), EPS)
    dn8 = (qd / nrm * 64.0).astype(NP_FP8)  # [B', N', D]
    cn = cc / np.maximum(np.linalg.norm(cc, axis=1, keepdims=True), EPS)
    cnT64 = np.ascontiguousarray(cn.T * 64.0).astype(NP_FP8)  # [D, K]
    cnt2 = np.ascontiguousarray(
        cnT64.reshape(CP, 2, P, K).transpose(0, 2, 1, 3)
    ).reshape(CP * P, 2 * K)
    cen16 = cc.astype(NP_BF16)
    ident = np.eye(K, dtype=NP_BF16)
    in_maps = []
    for core in range(ncores):
        sh = dn8[core * imgs:(core + 1) * imgs, :NN]  # [imgs, NN, D]
        # nat row (b, s, p) = [cp, q, d] flat (6KB contiguous per packet)
        nat = np.ascontiguousarray(
            sh.reshape(imgs, S, pps, 2, P, D).transpose(0, 1, 4, 2, 3, 5)
        ).reshape(imgs * S * P, pps * 2 * D)
        # tsp row (b, s, p) = [c, q, n] flat with (c,q,n) = desc[b, n0+n, 256c+128q+p]
        shT = sh.transpose(0, 2, 1)  # [imgs, D, NN]
        tsp = np.ascontiguousarray(
            shT.reshape(imgs, CP, 2, P, S, nsl).transpose(0, 4, 3, 1, 2, 5)
        ).reshape(imgs * S * P, CP * 2 * nsl)
        in_maps.append({
            "descsn": nat,
            "descst": tsp,
            "cnt2": cnt2,
            "cen": cen16,
            "ident": ident,
        })
    return in_maps


_NC_CACHE = {}


def _get_nc():
    if "nc" not in _NC_CACHE:
        _NC_CACHE["nc"] = build_nc()
    return _NC_CACHE["nc"]


def kernel(query_descs, c_centers):
    in_maps = prep_inputs(query_descs, c_centers)
    nc = _get_nc()
    res = run_bass_kernel_spmd(nc, in_maps, core_ids=list(range(NCORES)))
    out = np.concatenate(
        [res.results[i]["out"] for i in range(NCORES)], axis=0
    )  # [B, K*D] bf16
    return out.astype(np.float32)


# revision 21
# speedup vs baseline: 1.0427x; 1.0427x over previous
"""AnyLoc VLAD (vq_codebook) Trainium2 kernel, 8-core data parallel. v3.

Reference computation (per image, N=1024 patches, K=64 clusters, D=1536):
  descs_n = l2norm(query_descs)                 # row-normalize descriptors
  labels  = argmax_k(descs_n . l2norm(centers)) # hard assignment
  sum_d_k = sum_{n: label=k} descs_n            # per-cluster sum
  un_vlad = sum_d_k - count_k * centers_k
  vlad    = l2norm_rows(un_vlad); flatten; l2norm

Sharding: data-parallel over the batch axis, 4 images per NeuronCore; each
core holds the whole (tiny) codebook; host concatenates the per-core
outputs (no collectives needed).

The kernel is DMA-wire-bound (12.6 MB of fp8 descriptors per core at
~330 GB/s aggregate across the 16 DMA engines), so the structure keeps the
two hardware-DGE queues (sync + scalar) streaming continuously and hides
all compute under them:

  - host pre-normalizes descriptors (fp8 at x64 scale) and ships TWO
    layouts: natural patch-major pair tiles (agg rhs) and a DoubleRow-
    packed transposed layout (sims rhs). Every DMA row is one contiguous
    6 KB per-partition packet.
  - the pipeline runs in HALF-IMAGE slots (512 patches): per slot one tsp
    DMA (sync queue) + one nat DMA (scalar queue), 6 DoubleRow fp8 sims
    matmuls (codebook stationary -> simsT [64,512] psum, 1 bank), an ACT
    copy to bf16, 4 PE transposes back to patch-major (one shared bank;
    `start` only on the first write so a lazy hardware zero cannot wipe
    earlier chunks), one segmented DVE row-max and one is_ge against a
    stride-0 broadcast -> exact 1.0 one-hot fp8.
  - each slot's aggregation (6 DoubleRow matmuls + 2 one-column counts
    matmuls, accumulated per image) is deferred into the NEXT slot's
    stream so TensorE alternates sims_s+1 / agg_s with no idle window.
  - finalize: un_vlad = (-64*counts)*centers + agg in one pass split
    halves across DVE and GPSIMD (reads the agg psum directly, freeing
    banks); row norms via ACT Square+accum; global norm = sqrt(#nonzero
    rows) via a ones-matmul; final scale split ACT/GPSIMD; bf16 output
    (host upcasts) halves the write traffic; out DMAs ride the sync queue
    whose input traffic ends first.
  - PSUM budget exactly 8 banks: 2 sims + 2 transpose + 1 counts + 3 agg.

Toolchain workarounds: this walrus build accepts only one sync wait per
instruction, so Tile's tail drain is re-spread across per-engine drains
and a post-pass hoists surplus waits onto no-op carriers.
"""

import os
import sys

import numpy as np

for _p in ("/opt/trn_rl_repo", "/root/.axon_site/_ro/trn_rl_repo"):
    if os.path.isdir(_p) and _p not in sys.path:
        sys.path.insert(0, _p)

from contextlib import ExitStack

import ml_dtypes
import bass_rust
import concourse.bass as bass
import concourse.tile as tile
from concourse import bass_isa, library_config, mybir
from concourse.bass_utils import run_bass_kernel_spmd

B, N, K, D = 32, 1024, 64, 1536
NCORES = 8
IMGS = B // NCORES  # images per core
P = 128
NPAIR = 4   # patch chunk-pairs per image (N = NPAIR*256)
CP = 6      # feature chunk-pairs (D = CP*256)
JJ = D // 512  # agg column blocks
DH = D // 2    # finalize half split
BF16 = mybir.dt.bfloat16
FP8 = mybir.dt.float8e4
F32 = mybir.dt.float32
NP_BF16 = ml_dtypes.bfloat16
NP_FP8 = ml_dtypes.float8_e4m3
Alu = mybir.AluOpType
Act = mybir.ActivationFunctionType
DR = mybir.MatmulPerfMode.DoubleRow
EPS = 1e-12


def _patch_tile_drain():
    """This walrus build only accepts ONE sync wait per instruction; Tile's
    tail drain aggregates every outstanding semaphore wait onto a single
    Drain. Spread the waits across extra per-engine drains (all still
    before the end-of-kernel barrier, so semantics are unchanged)."""
    if getattr(tile.TileContext, "_vlad_drain_patched", False):
        return
    from concourse.vector_clock import ScopedClock

    def patched(self, tick_clock, wait_clock):
        nc = self.nc
        probe = nc.sync.drain()
        wait_clock.add_sem_waits(
            probe.ins, ScopedClock({None: tick_clock.global_clock})
        )
        si = probe.ins.sync_info
        waits = list(si.on_wait) if si is not None else []
        upds = list(si.on_update) if si is not None else []
        probe.ins.sync_info = bass_rust.SyncInfo(on_wait=waits[:1], on_update=upds)
        engines = [nc.scalar, nc.vector, nc.tensor, nc.gpsimd, nc.sync]
        for i, w in enumerate(waits[1:]):
            d = engines[i % len(engines)].drain()
            dsi = d.ins.sync_info
            du = list(dsi.on_update) if dsi is not None else []
            d.ins.sync_info = bass_rust.SyncInfo(on_wait=[w], on_update=du)
        nc.all_engine_barrier()
        popped = nc._tile_sem_poison_stack.pop()
        assert popped is self._sem_poison
        nc.clear_and_free_semaphores(list(self.sems.allocated().values()))

    tile.TileContext._drain_and_barrier = patched
    tile.TileContext._vlad_drain_patched = True


def _split_multi_waits(nc):
    """Walrus here accepts only one sync wait per instruction. Hoist surplus
    waits onto no-op carrier instructions inserted just before, on the same
    engine (safe: same engine executes in order, so all waits still complete
    before the original instruction issues)."""
    n_new = 0
    for _bbname, bassbb in list(nc.bb_map.items()):
        bb = bassbb.bb
        out = []
        changed = False
        for ins in bb.instructions:
            si = getattr(ins, "sync_info", None)
            waits = list(si.on_wait) if si is not None else []
            if len(waits) > 1:
                changed = True
                for w in waits[:-1]:
                    n_new += 1
                    nop = mybir.InstNoOp(
                        name=f"{ins.name}-wsplit{n_new}",
                        sync_info=mybir.SyncInfo(on_wait=[w], on_update=[]),
                        bass_nofuse=True,
                        engine=ins.engine,
                    )
                    nc.register_instruction(nop)
                    out.append(nop)
                ins.sync_info = bass_rust.SyncInfo(
                    on_wait=[waits[-1]], on_update=list(si.on_update)
                )
            out.append(ins)
        if changed:
            bb.instructions = out
    return n_new


def _slot_geom(npair):
    """Half-image slotting: S slots per image, pps chunk-pairs per slot."""
    S = 2 if npair % 2 == 0 and npair >= 2 else 1
    pps = npair // S
    nsl = pps * 2 * P  # patch columns per slot
    return S, pps, nsl


def build_nc(imgs=IMGS, npair=NPAIR):
    """Build the per-core Bass graph. `imgs`/`npair` shrinkable for sim."""
    _patch_tile_drain()
    S, pps, nsl = _slot_geom(npair)
    nch_s = 2 * pps  # 128-patch chunks per slot

    nc = bass.Bass("TRN2", target_bir_lowering=False, debug=False)
    # natural pair tiles: row (slot, p) = 6KB [cp, q, d] flat, where
    # element (cp, q, d) = desc[chunk 2*(slot_pairbase+cp)+q, patch p, d]
    descsn_e = nc.dram_tensor("descsn", [imgs * S * P, pps * 2 * D], FP8,
                              kind="ExternalInput")
    # DoubleRow-packed transpose: row (slot, p) = 6KB [c, q, n] flat with
    # element (c, q, n) = desc[b, slot_n0 + n, 256c+128q+p]
    descst_e = nc.dram_tensor("descst", [imgs * S * P, CP * 2 * nsl], FP8,
                              kind="ExternalInput")
    # codebook, same DoubleRow packing: row (c, p) = [q, k] = cnorm64[k, 256c+128q+p]
    cnt2_e = nc.dram_tensor("cnt2", [CP * P, 2 * K], FP8, kind="ExternalInput")
    cen_e = nc.dram_tensor("cen", [K, D], BF16, kind="ExternalInput")
    ident_e = nc.dram_tensor("ident", [K, K], BF16, kind="ExternalInput")
    out_e = nc.dram_tensor("out", [imgs, K * D], BF16, kind="ExternalOutput")

    with tile.TileContext(nc) as tc:
        with ExitStack() as ctx:
            consts = ctx.enter_context(tc.tile_pool(name="consts", bufs=1))
            tspp = ctx.enter_context(tc.tile_pool(name="tspp", bufs=6))
            natp = ctx.enter_context(tc.tile_pool(name="natp", bufs=6))
            simsbp = ctx.enter_context(tc.tile_pool(name="simsbp", bufs=2))
            asnp = ctx.enter_context(tc.tile_pool(name="asnp", bufs=2))
            mxp = ctx.enter_context(tc.tile_pool(name="mxp", bufs=2))
            uvp = ctx.enter_context(tc.tile_pool(name="uvp", bufs=2))
            sqp = ctx.enter_context(tc.tile_pool(name="sqp", bufs=2))
            vfinp = ctx.enter_context(tc.tile_pool(name="vfinp", bufs=2))
            finp = ctx.enter_context(tc.tile_pool(name="finp", bufs=16))
            simsps = ctx.enter_context(
                tc.tile_pool(name="simsps", bufs=2, space="PSUM"))
            transps = ctx.enter_context(
                tc.tile_pool(name="transps", bufs=2, space="PSUM"))
            cntps = ctx.enter_context(
                tc.tile_pool(name="cntps", bufs=1, space="PSUM"))
            aggps = ctx.enter_context(
                tc.tile_pool(name="aggps", bufs=1, space="PSUM"))

            cnt_sb = consts.tile([P, CP, 2, K], FP8)
            nc.sync.dma_start(
                out=cnt_sb,
                in_=cnt2_e.ap().rearrange("(c p) (q k) -> p c q k", c=CP, q=2),
            )
            ident_sb = consts.tile([K, K], BF16)
            nc.sync.dma_start(out=ident_sb, in_=ident_e.ap())
            onesc = consts.tile([P, 2, 1], FP8)
            nc.vector.memset(onesc, 1.0)
            # finalize-only consts load lazily in slot 0's shadow
            late = {}

            def _late():
                if not late:
                    cen_sb = consts.tile([K, D], BF16)
                    nc.scalar.dma_start(out=cen_sb, in_=cen_e.ap())
                    onesg = consts.tile([K, K], BF16)
                    nc.vector.memset(onesg, 1.0)
                    late.update(cen_sb=cen_sb, onesg=onesg)
                return late

            img_agg = {}

            def emit_agg(s):
                """Aggregate slot s's one-hot (deferred one slot)."""
                b, sj = s["b"], s["sj"]
                if sj == 0:
                    img_agg[b] = dict(
                        agg=aggps.tile([K, JJ, 512], F32, tag="agg", name="agg"),
                        counts=cntps.tile([K, 1], F32, tag="cnt", name="counts"),
                    )
                ia = img_agg[b]
                for cp in range(pps):
                    for jj in range(JJ):
                        nc.tensor.matmul(
                            ia["agg"][:, jj, :],
                            lhsT=s["asn"][:, 2 * cp:2 * cp + 2, :],
                            rhs=s["nat"][:, cp, :, jj * 512:(jj + 1) * 512],
                            start=(sj == 0 and cp == 0), stop=False,
                            perf_mode=DR, skip_group_check=True,
                        )
                for cp in range(pps):
                    nc.tensor.matmul(
                        ia["counts"], lhsT=s["asn"][:, 2 * cp:2 * cp + 2, :],
                        rhs=onesc,
                        start=(sj == 0 and cp == 0),
                        stop=(sj == S - 1 and cp == pps - 1),
                        perf_mode=DR, skip_group_check=True,
                    )

            def emit_fin1(b):
                """Image finalize part 1: un_vlad + row norm accumulation.
                GPSIMD cannot touch PSUM, so the -64*counts*centers term is
                folded into the agg psum by a diagonal-lhsT matmul (TensorE
                has slack in this DMA-bound regime). un_vlad is then copied
                to SBUF in ACT/DVE halves, releasing the agg banks early so
                the next image's aggregation never stalls on this finalize;
                the row-norm accumulation also runs as parallel halves."""
                lc = _late()
                ia = img_agg.pop(b)
                # diag = ident * counts * -64 in ONE DVE op (fewer hops in
                # the drain-critical finalize chain)
                diag = finp.tile([K, K], BF16, tag="diag")
                nc.vector.tensor_scalar(
                    diag, ident_sb, scalar1=ia["counts"], scalar2=-64.0,
                    op0=Alu.mult, op1=Alu.mult)
                for jj in range(JJ):
                    nc.tensor.matmul(
                        ia["agg"][:, jj, :], lhsT=diag,
                        rhs=lc["cen_sb"][:, jj * 512:(jj + 1) * 512],
                        start=False, stop=(jj == JJ - 1),
                        skip_group_check=True,
                    )
                aggflat = ia["agg"].rearrange("k a b -> k (a b)")
                uv = uvp.tile([K, D], F32, tag="uv")
                nc.scalar.mul(uv[:, 0:DH], aggflat[:, 0:DH], 1.0)
                nc.vector.tensor_scalar_mul(uv[:, DH:D], aggflat[:, DH:D], 1.0)
                sq = sqp.tile([K, D], FP8, tag="sq")
                r2a = finp.tile([K, 1], F32, tag="r2a")
                nc.scalar.activation(sq[:, 0:DH], uv[:, 0:DH], Act.Square,
                                     accum_out=r2a)
                r2b = finp.tile([K, 1], F32, tag="r2b")
                nc.vector.scalar_tensor_tensor(
                    out=sq[:, DH:D], in0=uv[:, DH:D], scalar=1.0,
                    in1=uv[:, DH:D], op0=Alu.mult, op1=Alu.mult,
                    accum_out=r2b,
                )
                r2 = finp.tile([K, 1], F32, tag="r2")
                nc.vector.tensor_tensor(r2, r2a, r2b, op=Alu.add)
                u = finp.tile([K, 1], F32, tag="u")
                nc.scalar.sqrt(u, r2)
                # nonzero-row gate from r2 (not u): lets the global-norm
                # branch (g/sg/ginv) run in parallel with um/invu
                sgate = finp.tile([K, 1], BF16, tag="sgate")
                nc.vector.tensor_scalar(
                    sgate, r2, scalar1=1e30, scalar2=1.0,
                    op0=Alu.mult, op1=Alu.min,
                )
                g_ps = transps.tile([K, 1], F32, tag="tr", name="g_ps")
                nc.tensor.matmul(g_ps, lhsT=lc["onesg"], rhs=sgate,
                                 start=True, stop=True, skip_group_check=True)
                sg = finp.tile([K, 1], F32, tag="sg")
                nc.scalar.sqrt(sg, g_ps)
                return dict(b=b, uv=uv, u=u, sg=sg)

            def emit_fin2(f):
                """Image finalize part 2: scales + output DMA."""
                um = finp.tile([K, 1], F32, tag="um")
                nc.vector.tensor_scalar_max(um, f["u"], EPS)
                invu = finp.tile([K, 1], F32, tag="invu")
                nc.vector.reciprocal(invu, um)
                ginv = finp.tile([K, 1], F32, tag="ginv")
                nc.vector.reciprocal(ginv, f["sg"])
                tot = finp.tile([K, 1], F32, tag="tot")
                nc.vector.tensor_mul(tot, invu, ginv)
                # final scale from the SBUF un_vlad, halves on ACT/DVE;
                # each half's output DMA starts as soon as it is ready
                vfin = vfinp.tile([K, D], BF16, tag="vfin")
                out_kd = out_e.ap()[f["b"]].rearrange("(k d) -> k d", k=K)
                nc.scalar.mul(vfin[:, 0:DH], f["uv"][:, 0:DH], tot)
                nc.sync.dma_start(out=out_kd[:, 0:DH], in_=vfin[:, 0:DH])
                nc.vector.tensor_scalar_mul(
                    vfin[:, DH:D], f["uv"][:, DH:D], tot)
                nc.sync.dma_start(out=out_kd[:, DH:D], in_=vfin[:, DH:D])

            # The PE drops to a low p-state after any idle gap and needs
            # ~3us of continuous execution to reach full clock, so the slot
            # pipeline is staged so that EVERY tensor instruction's inputs
            # are ready before the engine reaches it: transposes run one
            # slot behind sims (their ACT copy finished last stream), and
            # aggregation runs two slots behind (its one-hot finished last
            # stream). TensorE then never waits mid-run.
            def emit_transp_assign(s):
                trT = transps.tile([P, nch_s, K], BF16, tag="tr", name="trT")
                for ch in range(nch_s):
                    nc.tensor.matmul(
                        trT[:, ch, :],
                        lhsT=s["simsSb"][:, ch * P:(ch + 1) * P],
                        rhs=ident_sb, is_transpose=True,
                        start=(ch == 0), stop=(ch == nch_s - 1),
                        skip_group_check=True,
                    )
                mx = mxp.tile([P, nch_s], F32, tag="mx", name="mx")
                nc.vector.tensor_reduce(
                    mx, trT, axis=mybir.AxisListType.X, op=Alu.max)
                asn = asnp.tile([P, nch_s, K], FP8, tag="asn", name="asn")
                nc.vector.scalar_tensor_tensor(
                    out=asn, in0=trT, scalar=1.0,
                    in1=mx[:, :, None].broadcast_to([P, nch_s, K]),
                    op0=Alu.mult, op1=Alu.is_ge,
                )
                s["asn"] = asn

            prev1 = None  # slot awaiting transpose+assign (1 behind)
            prev2 = None  # slot awaiting aggregation (2 behind)
            fin1_pending = None
            for t in range(imgs * S):
                b, sj = divmod(t, S)
                tsp = tspp.tile([P, CP, 2, nsl], FP8, tag="tsp")
                nc.sync.dma_start(
                    out=tsp,
                    in_=descst_e.ap()[t * P:(t + 1) * P, :]
                    .rearrange("p (c q n) -> p c q n", c=CP, q=2),
                )
                nat = natp.tile([P, pps, 2, D], FP8, tag="nat")
                nc.scalar.dma_start(
                    out=nat,
                    in_=descsn_e.ap()[t * P:(t + 1) * P, :]
                    .rearrange("p (c q d) -> p c q d", c=pps, q=2),
                )
                # sims^T: codebook-stationary DoubleRow accumulation
                simsT = simsps.tile([K, nsl], F32, tag="sims")
                for c in range(CP):
                    nc.tensor.matmul(
                        simsT, lhsT=cnt_sb[:, c], rhs=tsp[:, c],
                        start=(c == 0), stop=(c == CP - 1),
                        perf_mode=DR, skip_group_check=True,
                    )
                simsSb = simsbp.tile([K, nsl], BF16, tag="simsb")
                nc.scalar.mul(simsSb, simsT, 1.0)
                if prev1 is not None:
                    emit_transp_assign(prev1)
                if prev2 is not None:
                    emit_agg(prev2)
                    if prev2["sj"] == S - 1:
                        fin1_pending = emit_fin1(prev2["b"])
                # fin2 at stream end: the fin1 ACT round-trip has completed
                # and the agg banks free before the next image rotates in
                if fin1_pending is not None and (
                        prev2 is None or prev2["sj"] == S - 1):
                    emit_fin2(fin1_pending)
                    fin1_pending = None
                prev2 = prev1
                prev1 = dict(b=b, sj=sj, nat=nat, simsSb=simsSb)
            # drain: transpose/assign of the last slot, then the last two
            # slots' aggregation and the final image's finalize
            emit_transp_assign(prev1)
            if prev2 is not None:
                emit_agg(prev2)
                if prev2["sj"] == S - 1:
                    emit_fin2(emit_fin1(prev2["b"]))
            emit_agg(prev1)
            if prev1["sj"] == S - 1:
                emit_fin2(emit_fin1(prev1["b"]))

    _split_multi_waits(nc)
    return nc


def prep_inputs(query_descs, c_centers, imgs=IMGS, npair=NPAIR, ncores=NCORES):
    """Host-side layout prep shared by kernel() and tests."""
    S, pps, nsl = _slot_geom(npair)
    NN = npair * 2 * P
    qd = np.ascontiguousarray(query_descs, dtype=np.float32)
    cc = np.ascontiguousarray(c_centers, dtype=np.float32)
    # normalized descriptors at x64 scale (sweet spot for fp8e4m3); the
    # x64 factors cancel in argmax and under the downstream l2norms
    nrm = np.maximum(np.linalg.norm(qd, axis=-1, keepdims=True), EPS)
    dn8 = (qd / nrm * 64.0).astype(NP_FP8)  # [B', N', D]
    cn = cc / np.maximum(np.linalg.norm(cc, axis=1, keepdims=True), EPS)
    cnT64 = np.ascontiguousarray(cn.T * 64.0).astype(NP_FP8)  # [D, K]
    cnt2 = np.ascontiguousarray(
        cnT64.reshape(CP, 2, P, K).transpose(0, 2, 1, 3)
    ).reshape(CP * P, 2 * K)
    cen16 = cc.astype(NP_BF16)
    ident = np.eye(K, dtype=NP_BF16)
    in_maps = []
    for core in range(ncores):
        sh = dn8[core * imgs:(core + 1) * imgs, :NN]  # [imgs, NN, D]
        # nat row (b, s, p) = [cp, q, d] flat (6KB contiguous per packet)
        nat = np.ascontiguousarray(
            sh.reshape(imgs, S, pps, 2, P, D).transpose(0, 1, 4, 2, 3, 5)
        ).reshape(imgs * S * P, pps * 2 * D)
        # tsp row (b, s, p) = [c, q, n] flat with (c,q,n) = desc[b, n0+n, 256c+128q+p]
        shT = sh.transpose(0, 2, 1)  # [imgs, D, NN]
        tsp = np.ascontiguousarray(
            shT.reshape(imgs, CP, 2, P, S, nsl).transpose(0, 4, 3, 1, 2, 5)
        ).reshape(imgs * S * P, CP * 2 * nsl)
        in_maps.append({
            "descsn": nat,
            "descst": tsp,
            "cnt2": cnt2,
            "cen": cen16,
            "ident": ident,
        })
    return in_maps


_NC_CACHE = {}


def _get_nc():
    if "nc" not in _NC_CACHE:
        _NC_CACHE["nc"] = build_nc()
    return _NC_CACHE["nc"]


def kernel(query_descs, c_centers):
    in_maps = prep_inputs(query_descs, c_centers)
    nc = _get_nc()
    res = run_bass_kernel_spmd(nc, in_maps, core_ids=list(range(NCORES)))
    out = np.concatenate(
        [res.results[i]["out"] for i in range(NCORES)], axis=0
    )  # [B, K*D] bf16
    return out.astype(np.float32)
